# revision 4
# baseline (speedup 1.0000x reference)
"""Trainium2 Bass kernel for CLSControlledDynamicBlock — single-launch v2.

Computation (per reference):
  x = cls_token[:, 0, :]                      # (16, 768)
  h = relu(x @ W1 + b1)                       # (16, 192)
  params = tanh(h @ W2 + b2)                  # (16, 36864)
  w = params.reshape(16, 64, 64, 3, 3)        # per-sample conv kernels
  out[s] = conv2d_same(features[s], w[s]) + features[s]

The MLP is 0.23% of the FLOPs; it and all layout work run on host.  The
device does ONE SPMD launch: the per-sample conv, 2 samples per core.

Device layout per core (samples A, B):
  planes pf[0] = (A-F | A-G), pf[1] = (B-G | B-F) where F is the padded
  bf16 image with channel on the partition dim and G is F shifted one
  row down — so a K=128 matmul computes the ky=0 and ky=1 taps at once
  (3 kx slots), and the ky=2 taps run as K=64 singles on the F halves
  in concurrent PE quadrants ((0,0) for A, (64,64) for B).
  The identity shortcut is folded into the (ky=1, kx=1) pair weight on
  host, so PSUM accumulates conv + residual directly.  Work is
  pipelined in row bands; per 4-row PSUM chunk: 12 matmuls, then one
  full-width PSUM->SBUF bf16 copy (alternating DVE / ACT to split the
  PSUM read-port load), streamed to HBM in 8-row out-DMAs (bands
  alternate the SP / ACT rings).  Host upcasts the bf16 result to f32.
"""

import numpy as np
import ml_dtypes

import concourse.mybir as mybir
import concourse.tile as tile
from concourse import bacc
from concourse.bass_utils import run_bass_kernel_spmd

F32 = mybir.dt.float32
BF16 = mybir.dt.bfloat16
AF = mybir.ActivationFunctionType

B, EMB, CIN, COUT, K, H, W = 16, 768, 64, 64, 3, 112, 112
HID = EMB // 4  # 192
TOTAL = COUT * CIN * K * K  # 36864
NCORES = 8

HP = W + 2  # 114 padded width
HPAD = H + 4  # 116 plane rows
CH = 4  # output rows per PSUM chunk

# Output-row bands: geometric ramp-up so PE work tracks DMA arrival
# (~0.15us/row to load vs ~0.30us/row to compute), with a tiny last
# band to cut the kernel tail.
BANDS = [(0, 8), (8, 8), (16, 12), (28, 16), (44, 20), (64, 24), (88, 16),
         (104, 8)]
NBD = len(BANDS)
NSPLIT = 2  # leading bands whose plane DMAs split across both rings
NWARM = 24  # junk matmuls to warm the PE clock before band 0 lands


def build_conv():
    nc = bacc.Bacc("TRN2", target_bir_lowering=False, debug=False,
                   num_devices=NCORES)
    pf = nc.dram_tensor("pf", [2, 128, HPAD, HP], BF16, kind="ExternalInput")
    # Pair weights wp[p, s, kx, co]: sample A partitions (ky=0 ci|ky=1 ci),
    # sample B flipped (ky=1 | ky=0), matching the plane layout. ws holds
    # the ky=2 taps: partitions (A ci | B ci).
    wp = nc.dram_tensor("wp", [128, 2, K, COUT], BF16, kind="ExternalInput")
    ws = nc.dram_tensor("ws", [128, K, COUT], BF16, kind="ExternalInput")
    out = nc.dram_tensor("out", [2, COUT, H, W], BF16, kind="ExternalOutput")
    outp = out.ap().rearrange("s c r x -> (s c) r x")

    with tile.TileContext(nc) as tc:
        with (
            tc.tile_pool(name="const", bufs=1) as const,
            tc.tile_pool(name="bands", bufs=1) as bands,
            tc.tile_pool(name="outs", bufs=2) as outs,
            tc.tile_pool(name="psum", bufs=1, space="PSUM") as psum,
        ):
            # Weights ride the gpsimd SWDGE ring: this delays the
            # TileContext start barrier ~2us (PE starts late) but keeps
            # both HWDGE rings clear for band planes, so the PE, once
            # running, never waits on a plane.
            wpair = const.tile([128, 2, K, COUT], BF16, tag="wpair")
            nc.gpsimd.dma_start(wpair[:], wp.ap())
            wsing = const.tile([128, K, COUT], BF16, tag="wsing")
            nc.gpsimd.dma_start(wsing[:], ws.ap())
            plA, plB = [], []
            for bnd, (s0, n) in enumerate(BANDS):
                PR = n + 3
                a = bands.tile([128, PR, HP], BF16, tag=f"plA{bnd}",
                               name=f"plA{bnd}")
                bb = bands.tile([128, PR, HP], BF16, tag=f"plB{bnd}",
                                name=f"plB{bnd}")
                if bnd < NSPLIT:
                    r1 = PR // 2
                    nc.sync.dma_start(a[:, 0:r1], pf.ap()[0, :, s0:s0 + r1, :])
                    nc.scalar.dma_start(a[:, r1:PR],
                                        pf.ap()[0, :, s0 + r1:s0 + PR, :])
                    nc.scalar.dma_start(bb[:, 0:r1],
                                        pf.ap()[1, :, s0:s0 + r1, :])
                    nc.sync.dma_start(bb[:, r1:PR],
                                      pf.ap()[1, :, s0 + r1:s0 + PR, :])
                else:
                    nc.sync.dma_start(a[:], pf.ap()[0, :, s0:s0 + PR, :])
                    nc.scalar.dma_start(bb[:], pf.ap()[1, :, s0:s0 + PR, :])
                plA.append(a)
                plB.append(bb)

            # PE warm-up: junk matmuls so HAM is at full clock by the
            # time the first real matmul issues.
            junk = const.tile([128, 128], BF16, tag="junk")
            nc.gpsimd.memset(junk[:], 0.0)
            jps = psum.tile([128, CH, W], F32, tag="ps", bufs=8, name="jps")
            for i in range(NWARM):
                nc.tensor.matmul(jps.rearrange('p r c -> p (r c)')[:, 0:128],
                                 junk[:], junk[:],
                                 start=(i == 0), stop=(i == NWARM - 1),
                                 skip_group_check=True)

            for bnd, (s0, n) in enumerate(BANDS):
                cpb = n // CH
                ob = outs.tile([128, n, W], BF16, tag=f"ob{bnd % 2}",
                               name=f"ob{bnd}")
                pss = [psum.tile([128, CH, W], F32, tag="ps", bufs=8,
                                 name=f"ps{bnd}_{j}") for j in range(cpb)]
                for t in range(2 * K):  # 3 pair slots then 3 single slots
                    kx = t % K
                    for j in range(cpb):
                        for s in range(2):
                            sl = slice(s * 64, (s + 1) * 64)
                            pl = (plA, plB)[s][bnd]
                            if t < K:  # ky={0,1} pair, K=128
                                lhsT = wpair[:, s, kx, :]
                                rhs = pl[:, CH * j:CH * j + CH, kx:kx + W]
                            else:  # ky=2 single, K=64 on the F plane
                                lhsT = wsing[sl, kx, :]
                                rhs = pl[sl, CH * j + 2:CH * j + 2 + CH,
                                         kx:kx + W]
                            nc.tensor.matmul(
                                pss[j][sl], lhsT, rhs,
                                start=(t == 0), stop=(t == 2 * K - 1),
                                tile_position=(0 if t < K else s * 64,
                                               s * 64),
                                skip_group_check=True)
                dma_eng = nc.sync if bnd % 2 == 0 else nc.scalar
                for j in range(cpb):
                    # PSUM already holds conv + residual (identity folded
                    # into the ky=1,kx=1 pair weight on host). PSUM->SBUF
                    # copies alternate DVE / ACT: each engine's PSUM read
                    # port tops out near ~370 GB/s, so splitting the 6.4MB
                    # of PSUM drain across both halves the copy wall.
                    lj = CH * j
                    if j % 2 == 0:
                        nc.vector.tensor_copy(
                            out=ob[:, lj:lj + CH, :], in_=pss[j][:])
                    else:
                        nc.scalar.activation(
                            ob[:, lj:lj + CH, :], pss[j][:], AF.Copy)
                    # stream the band out in pairs of chunks to keep the
                    # out-queue busy through the whole kernel
                    if j % 2 == 1:
                        y0 = s0 + lj
                        dma_eng.dma_start(
                            outp[:, y0 - CH:y0 + CH, :],
                            ob[:, lj - CH:lj + CH, :])
                if cpb % 2 == 1:
                    dma_eng.dma_start(
                        outp[:, s0 + n - CH:s0 + n, :],
                        ob[:, n - CH:n, :])

    nc.compile()
    return nc


def _mlp_params(cls_token, W1, b1, W2, b2):
    x = cls_token[:, 0, :].astype(np.float32)
    h = np.maximum(x @ W1 + b1, 0.0)
    return np.tanh(h @ W2 + b2)  # (B, TOTAL) f32


def _weight_slabs(params):
    # paramsT rows are (co, ci, ky, kx). Build per-core pair/single slabs:
    #   T[s, ky, ci, kx, co] = w[s][co, ci, ky, kx]
    # The identity shortcut folds into the (ky=1, kx=1) tap: residual =
    # feature[co, r, x] = P[co, r+1, x+1], which is exactly that tap's
    # input with an identity weight -- so PSUM accumulates conv+residual
    # and no separate add is needed.
    T = np.ascontiguousarray(
        params.T.reshape(COUT, CIN, K, K, B).transpose(4, 2, 1, 3, 0)
    ).astype(np.float32)
    eye = np.eye(CIN, COUT, dtype=np.float32)
    wps, wss = [], []
    for j in range(NCORES):
        A, Bm = T[2 * j], T[2 * j + 1]
        wpc = np.empty((128, 2, K, COUT), dtype=np.float32)
        wpc[:64, 0] = A[0]; wpc[64:, 0] = A[1]   # A: (F=ky0 | G=ky1)
        wpc[:64, 1] = Bm[1]; wpc[64:, 1] = Bm[0]  # B flipped: (G=ky1 | F=ky0)
        wpc[64:, 0, 1] += eye   # A residual via G half (ky=1), kx=1
        wpc[:64, 1, 1] += eye   # B residual via G half (ky=1), kx=1
        wsc = np.empty((128, K, COUT), dtype=np.float32)
        wsc[:64] = A[2]; wsc[64:] = Bm[2]
        wps.append(np.ascontiguousarray(wpc.astype(ml_dtypes.bfloat16)))
        wss.append(np.ascontiguousarray(wsc.astype(ml_dtypes.bfloat16)))
    return wps, wss


def _planes(features):
    # Padded bf16 image planes, F and its one-row-down shift G,
    # pre-interleaved on the partition axis exactly as SBUF wants them.
    pad = np.zeros((B, CIN, HPAD, HP), dtype=ml_dtypes.bfloat16)
    pad[:, :, 1:1 + H, 1:1 + W] = features
    padG = np.zeros_like(pad)
    padG[:, :, 0:HPAD - 1] = pad[:, :, 1:HPAD]
    pfA = np.concatenate([pad[0::2], padG[0::2]], axis=1)   # (8,128,116,114)
    pfB = np.concatenate([padG[1::2], pad[1::2]], axis=1)
    return np.ascontiguousarray(np.stack([pfA, pfB], axis=1))


_cache = {}


def _get(name, builder):
    if name not in _cache:
        _cache[name] = builder()
    return _cache[name]


def prep_inputs(cls_token, features, W1, b1, W2, b2):
    params = _mlp_params(cls_token, W1, b1, W2, b2)
    wps, wss = _weight_slabs(params)
    pf = _planes(features)
    return [{"pf": pf[j], "wp": wps[j], "ws": wss[j]} for j in range(NCORES)]


def kernel(cls_token, features, W1, b1, W2, b2):
    cls_token = np.asarray(cls_token, dtype=np.float32)
    features = np.ascontiguousarray(np.asarray(features, dtype=np.float32))
    W1 = np.ascontiguousarray(np.asarray(W1, dtype=np.float32))
    b1 = np.asarray(b1, dtype=np.float32)
    W2 = np.asarray(W2, dtype=np.float32)
    b2 = np.asarray(b2, dtype=np.float32)

    ncC = _get("C", build_conv)
    in_maps = prep_inputs(cls_token, features, W1, b1, W2, b2)
    res = run_bass_kernel_spmd(ncC, in_maps, core_ids=list(range(NCORES)))
    out = np.concatenate(
        [res.results[j]["out"] for j in range(NCORES)], axis=0)
    return out.astype(np.float32)
